# revision 22
# baseline (speedup 1.0000x reference)
"""Multi-head attention (single head, d_model=1024) Bass kernel for 8 trn2 cores.

Problem: query/key/value [4, 4096, 1024] fp32 ->
    out = softmax(Q K^T / sqrt(1024)) V   per batch.

Sharding: pure data parallel. 2 cores per batch element; each core handles
2048 query rows against the batch's full K/V (sequence-parallel over query
rows with full K/V replication).

Per-core device kernel (all matmuls bf16 inputs, fp32 PSUM accumulation):
  inputs (host pre-transposed/cast): qT [1024, 2048], kT [1024, 4096],
  v [4096, 1024], all bf16.
  Phase 1 computes scores TRANSPOSED: sT[k, q] = sum_d kT[d,k] * qT[d,q],
  so that softmax weights land in [k, q] layout == exactly the lhsT layout
  the second matmul needs, and the output comes out in natural [q, d] layout.
  No max subtraction: scores ~ N(0,1) (scale 1/32 folded into ACT's
  free affine), exp cannot overflow.
  Denominators via an extra N=1 matmul against a ones column per q-panel:
  den[q] = sum_k exp[k, q]; final normalize on VectorE with per-partition
  reciprocal.
"""

import numpy as np
import ml_dtypes

P = 128
D = 1024
DC = D // P          # 8 contraction chunks
SK = 4096
NKT = SK // P        # 32 k-tiles
SQ = 2048            # query rows per core
SPQ = 512            # superpanel query width (phase-1 matmul free dim)
NSP = SQ // SPQ      # 4 superpanels
NQS = SPQ // P       # 4 q-subpanels per superpanel
KBLK = 512           # kT/v DMA block: 512 k rows (4 k-tiles)
NKB = SK // KBLK     # 8 dma blocks

_BUILT = {}


def _build_nc():
    import concourse.bass as bass
    import concourse.tile as tile
    from concourse import mybir

    bf16 = mybir.dt.bfloat16
    f32 = mybir.dt.float32

    nc = bass.Bass()
    qT = nc.declare_dram_parameter("qT", [D, SQ], bf16, isOutput=False)
    kT = nc.declare_dram_parameter("kT", [D, SK], bf16, isOutput=False)
    v = nc.declare_dram_parameter("v", [SK, D], bf16, isOutput=False)
    out = nc.declare_dram_parameter("out", [SQ, D], f32, isOutput=True)

    # DRAM views with the partition dim innermost-of-128:
    qT_r = qT[:].rearrange("(c p) q -> p c q", p=P)      # [128, 8, 2048]
    kT_r = kT[:].rearrange("(c p) k -> p c k", p=P)      # [128, 8, 4096]
    v_r = v[:].rearrange("(n p) d -> p n d", p=P)        # [128, 32, 1024]

    with tile.TileContext(nc) as tc:
        from contextlib import ExitStack

        with ExitStack() as ctx:
            wpool = ctx.enter_context(tc.tile_pool(name="wpool", bufs=1))
            qpool = ctx.enter_context(tc.tile_pool(name="qpool", bufs=4))
            expp = ctx.enter_context(tc.tile_pool(name="expp", bufs=NKT + 1))
            outp = ctx.enter_context(tc.tile_pool(name="outp", bufs=2))
            small = ctx.enter_context(tc.tile_pool(name="small", bufs=4))
            ps_s = ctx.enter_context(tc.tile_pool(name="ps_s", bufs=3, space="PSUM"))
            ps_o = ctx.enter_context(tc.tile_pool(name="ps_o", bufs=4, space="PSUM"))
            ps_d = ctx.enter_context(tc.tile_pool(name="ps_d", bufs=1, space="PSUM"))

            ones = wpool.tile([P, 1], bf16, tag="ones")
            nc.vector.memset(ones, 1.0)
            ones32 = wpool.tile([P, 1], f32, tag="ones32")
            nc.vector.memset(ones32, 1.0)
            accp = ctx.enter_context(tc.tile_pool(name="accp", bufs=2))

            # HAM warm-up: dummy back-to-back matmuls on a zeroed tile keep
            # the PE activity monitor busy while the first input DMAs land,
            # so the real matmuls start at 2.4 GHz instead of 1.2 GHz.
            warm = wpool.tile([P, 512], bf16, tag="warm")
            nc.vector.memset(warm, 0.0)
            wps = ps_s.tile([P, 512], f32, tag="ps")
            NWARM = 10
            for i in range(NWARM):
                nc.tensor.matmul(
                    wps, lhsT=warm[:, 0:P], rhs=warm,
                    start=(i == 0), stop=(i == NWARM - 1),
                )

            # First K^T block + first superpanel's Q first so PE can start
            # ~11us in, then the rest (phase-1 feed order), V trailing
            # (phase 2 only). qt0 split in half so the first d-chunk matmuls
            # start after 0.5 MB instead of 1 MB.
            qt_tiles = [
                qpool.tile([P, DC, SPQ], bf16, tag="qt", name=f"qt{i}")
                for i in range(NSP)
            ]
            kt_tiles = []
            vt_tiles = []
            for kb in range(NKB):
                kt_tiles.append(wpool.tile([P, DC, KBLK], bf16, tag=f"kT{kb}",
                                           name=f"kTs{kb}"))
                vt_tiles.append(wpool.tile([P, 4, D], bf16, tag=f"v{kb}",
                                           name=f"vs{kb}"))
            # Three DMA backends in parallel: K^T on SP-HWDGE, Q^T on
            # ACT-HWDGE, V on SWDGE. qpool bufs=4 keeps every qT load
            # wait-free so nothing can stall the ACT stream.
            nc.sync.dma_start(out=kt_tiles[0], in_=kT_r[:, :, 0:KBLK])
            nc.scalar.dma_start(out=qt_tiles[0][:, 0:4, :], in_=qT_r[:, 0:4, 0:SPQ])
            nc.scalar.dma_start(out=qt_tiles[0][:, 4:8, :], in_=qT_r[:, 4:8, 0:SPQ])
            for kb in range(1, NKB):
                nc.sync.dma_start(
                    out=kt_tiles[kb], in_=kT_r[:, :, kb * KBLK:(kb + 1) * KBLK]
                )
            for kb in range(NKB):
                nc.gpsimd.dma_start(out=vt_tiles[kb], in_=v_r[:, kb * 4:(kb + 1) * 4, :])
            for sp in range(1, NSP):
                nc.scalar.dma_start(
                    out=qt_tiles[sp], in_=qT_r[:, :, sp * SPQ:(sp + 1) * SPQ]
                )

            for sp in range(NSP):
                qt = qt_tiles[sp]

                # Phase 1: sT[k-tile, q] for all 32 k-tiles, then exp -> bf16.
                # VectorE keeps a running fp32 partition-wise sum of the exp
                # tiles so the softmax denominator needs only ONE tiny matmul
                # per q-subpanel instead of one per (q-subpanel, k-tile).
                exp_tiles = []
                acc = accp.tile([P, SPQ], f32, tag="acc", name=f"acc{sp}")
                for kt_i in range(NKT):
                    ps = ps_s.tile([P, SPQ], f32, tag="ps")
                    kb, ki = divmod(kt_i, 4)
                    for c in range(DC):
                        nc.tensor.matmul(
                            ps,
                            lhsT=kt_tiles[kb][:, c, ki * P:(ki + 1) * P],
                            rhs=qt[:, c, :],
                            start=(c == 0),
                            stop=(c == DC - 1),
                        )
                    e = expp.tile([P, SPQ], bf16, tag="exp")
                    nc.scalar.activation(
                        e, ps, mybir.ActivationFunctionType.Exp, scale=1.0 / 32.0
                    )
                    if kt_i == 0:
                        nc.vector.tensor_copy(out=acc, in_=e)
                    else:
                        nc.vector.tensor_add(acc, acc, e)
                    exp_tiles.append(e)

                # Phase 2: out[q, d] = exp^T @ V, per 128-query subpanel
                for qs in range(NQS):
                    po0 = ps_o.tile([P, 512], f32, tag="po")
                    po1 = ps_o.tile([P, 512], f32, tag="po")
                    pd = ps_d.tile([P, 1], f32, tag="pd")
                    for kt_i in range(NKT):
                        w = exp_tiles[kt_i][:, qs * P:(qs + 1) * P]
                        kb, ki = divmod(kt_i, 4)
                        st = kt_i == 0
                        fin = kt_i == NKT - 1
                        nc.tensor.matmul(
                            po0, lhsT=w, rhs=vt_tiles[kb][:, ki, 0:512],
                            start=st, stop=fin,
                        )
                        nc.tensor.matmul(
                            po1, lhsT=w, rhs=vt_tiles[kb][:, ki, 512:1024],
                            start=st, stop=fin,
                        )
                    # den[q] = sum_p acc[p, q] in one fp32 N=1 matmul
                    nc.tensor.matmul(
                        pd, lhsT=acc[:, qs * P:(qs + 1) * P], rhs=ones32[:, 0:1],
                        start=True, stop=True,
                    )
                    o = outp.tile([P, D], f32, tag="o")
                    # Wait-absorber: the TensorScalar ISA struct only fits one
                    # sem wait, so take the WAR-vs-previous-store wait on a
                    # cheap DVE memset first (WAW on o then orders via FIFO).
                    nc.vector.memset(o[:, 0:1], 0.0)
                    r = small.tile([P, 1], f32, tag="recip")
                    nc.vector.reciprocal(r, pd)
                    nc.vector.tensor_scalar_mul(o[:, 0:512], po0, r)
                    nc.vector.tensor_scalar_mul(o[:, 512:1024], po1, r)
                    row = (sp * NQS + qs) * P
                    # SWDGE: the HWDGE DIRECT2D pseudo only fits 2 sem waits
                    # and these stores need RAW(VE) + queue-order waits.
                    nc.gpsimd.dma_start(out=out[row:row + P, :], in_=o)

    _split_excess_waits(nc, mybir)
    return nc


def _split_excess_waits(nc, mybir):
    """This walrus build only fits ONE embedded sem wait per engine
    instruction (setupSyncWait: 'Too many sync wait commands'). Hoist all
    but one wait of each engine instruction onto standalone EventSemaphore
    instructions (the same thing wait_ge() emits) right before it in the
    same engine stream - semantically identical, engine streams are FIFO.
    SWDGE (Pool) DMACopy waits are executed by GPSIMD ucode and tolerate
    multiple waits; leave DMAs alone."""
    for fn in nc.m.functions:
        for bb in fn.blocks:
            new_insts = []
            changed = False
            for ins in bb.instructions:
                si = ins.sync_info
                if (
                    si is not None
                    and ins.opcode != "EventSemaphore"
                    and len(si.on_wait) > 1
                ):
                    waits = list(si.on_wait)
                    for j, w in enumerate(waits[:-1]):
                        new_insts.append(
                            mybir.InstEventSemaphore(
                                name=f"{ins.name}-xw{j}",
                                engine=ins.engine,
                                ins=[],
                                outs=[],
                                sync_info=mybir.SyncInfo(on_wait=[w], on_update=[]),
                            )
                        )
                    si.on_wait = [waits[-1]]
                    changed = True
                new_insts.append(ins)
            if changed:
                bb.instructions = new_insts


def get_nc():
    if "nc" not in _BUILT:
        _BUILT["nc"] = _build_nc()
    return _BUILT["nc"]


def kernel(query, key, value):
    from concourse.bass_utils import run_bass_kernel_spmd

    q = np.asarray(query)
    k = np.asarray(key)
    v = np.asarray(value)
    B, S, Dm = q.shape
    assert (B, S, Dm) == (4, 4096, 1024)
    bf = ml_dtypes.bfloat16

    in_maps = []
    for core in range(8):
        b, h = divmod(core, 2)
        qs = q[b, h * SQ:(h + 1) * SQ, :]
        in_maps.append({
            "qT": np.ascontiguousarray(qs.T).astype(bf),
            "kT": np.ascontiguousarray(k[b].T).astype(bf),
            "v": np.ascontiguousarray(v[b]).astype(bf),
        })

    nc = get_nc()
    res = run_bass_kernel_spmd(nc, in_maps, list(range(8)))

    out = np.empty((B, S, Dm), np.float32)
    for core in range(8):
        b, h = divmod(core, 2)
        out[b, h * SQ:(h + 1) * SQ, :] = res.results[core]["out"]
    return out


# revision 25
# speedup vs baseline: 1.0451x; 1.0451x over previous
"""Multi-head attention (single head, d_model=1024) Bass kernel for 8 trn2 cores.

Problem: query/key/value [4, 4096, 1024] fp32 ->
    out = softmax(Q K^T / sqrt(1024)) V   per batch.

Sharding: pure data parallel. 2 cores per batch element; each core handles
2048 query rows against the batch's full K/V (sequence-parallel over query
rows with full K/V replication).

Per-core device kernel (all matmuls bf16 inputs, fp32 PSUM accumulation):
  inputs (host pre-transposed/cast): qT [1024, 2048], kT [1024, 4096],
  v [4096, 1024], all bf16.
  Phase 1 computes scores TRANSPOSED: sT[k, q] = sum_d kT[d,k] * qT[d,q],
  so that softmax weights land in [k, q] layout == exactly the lhsT layout
  the second matmul needs, and the output comes out in natural [q, d] layout.
  No max subtraction: scores ~ N(0,1) (scale 1/32 folded into ACT's
  free affine), exp cannot overflow.
  Denominators via an extra N=1 matmul against a ones column per q-panel:
  den[q] = sum_k exp[k, q]; final normalize on VectorE with per-partition
  reciprocal.
"""

import numpy as np
import ml_dtypes

P = 128
D = 1024
DC = D // P          # 8 contraction chunks
SK = 4096
NKT = SK // P        # 32 k-tiles
SQ = 2048            # query rows per core
SPQ = 512            # superpanel query width (phase-1 matmul free dim)
NSP = SQ // SPQ      # 4 superpanels
NQS = SPQ // P       # 4 q-subpanels per superpanel
KBLK = 512           # kT/v DMA block: 512 k rows (4 k-tiles)
NKB = SK // KBLK     # 8 dma blocks

_BUILT = {}


def _build_nc():
    import concourse.bass as bass
    import concourse.tile as tile
    from concourse import mybir

    bf16 = mybir.dt.bfloat16
    f32 = mybir.dt.float32

    nc = bass.Bass()
    qT = nc.declare_dram_parameter("qT", [D, SQ], bf16, isOutput=False)
    kT = nc.declare_dram_parameter("kT", [D, SK], bf16, isOutput=False)
    v = nc.declare_dram_parameter("v", [SK, D], bf16, isOutput=False)
    out = nc.declare_dram_parameter("out", [SQ, D], f32, isOutput=True)

    # DRAM views with the partition dim innermost-of-128:
    qT_r = qT[:].rearrange("(c p) q -> p c q", p=P)      # [128, 8, 2048]
    kT_r = kT[:].rearrange("(c p) k -> p c k", p=P)      # [128, 8, 4096]
    v_r = v[:].rearrange("(n p) d -> p n d", p=P)        # [128, 32, 1024]

    with tile.TileContext(nc) as tc:
        from contextlib import ExitStack

        with ExitStack() as ctx:
            wpool = ctx.enter_context(tc.tile_pool(name="wpool", bufs=1))
            qpool = ctx.enter_context(tc.tile_pool(name="qpool", bufs=2))
            expp = ctx.enter_context(tc.tile_pool(name="expp", bufs=NKT + 1))
            outp = ctx.enter_context(tc.tile_pool(name="outp", bufs=3))
            small = ctx.enter_context(tc.tile_pool(name="small", bufs=4))
            ps_s = ctx.enter_context(tc.tile_pool(name="ps_s", bufs=3, space="PSUM"))
            ps_o = ctx.enter_context(tc.tile_pool(name="ps_o", bufs=4, space="PSUM"))
            ps_d = ctx.enter_context(tc.tile_pool(name="ps_d", bufs=1, space="PSUM"))

            ones = wpool.tile([P, 1], bf16, tag="ones")
            nc.vector.memset(ones, 1.0)
            ones32 = wpool.tile([P, 1], f32, tag="ones32")
            nc.vector.memset(ones32, 1.0)
            accp = ctx.enter_context(tc.tile_pool(name="accp", bufs=2))

            # HAM warm-up: dummy back-to-back matmuls on a zeroed tile keep
            # the PE activity monitor busy while the first input DMAs land,
            # so the real matmuls start at 2.4 GHz instead of 1.2 GHz.
            warm = wpool.tile([P, 512], bf16, tag="warm")
            nc.vector.memset(warm, 0.0)
            wps = ps_s.tile([P, 512], f32, tag="ps")
            NWARM = 14
            for i in range(NWARM):
                nc.tensor.matmul(
                    wps, lhsT=warm[:, 0:P], rhs=warm,
                    start=(i == 0), stop=(i == NWARM - 1),
                )

            # First K^T block + first superpanel's Q first so PE can start
            # ~11us in, then the rest (phase-1 feed order), V trailing
            # (phase 2 only). qt0 split in half so the first d-chunk matmuls
            # start after 0.5 MB instead of 1 MB.
            qt_tiles = [
                qpool.tile([P, DC, SPQ], bf16, tag="qt", name=f"qt{i}")
                for i in range(NSP)
            ]
            kt_tiles = []
            vt_tiles = []
            for kb in range(NKB):
                kt_tiles.append(wpool.tile([P, DC, KBLK], bf16, tag=f"kT{kb}",
                                           name=f"kTs{kb}"))
                vt_tiles.append(wpool.tile([P, 4, D], bf16, tag=f"v{kb}",
                                           name=f"vs{kb}"))
            nc.sync.dma_start(out=kt_tiles[0], in_=kT_r[:, :, 0:KBLK])
            nc.sync.dma_start(out=qt_tiles[0][:, 0:4, :], in_=qT_r[:, 0:4, 0:SPQ])
            nc.sync.dma_start(out=qt_tiles[0][:, 4:8, :], in_=qT_r[:, 4:8, 0:SPQ])
            for kb in range(1, NKB):
                nc.sync.dma_start(
                    out=kt_tiles[kb], in_=kT_r[:, :, kb * KBLK:(kb + 1) * KBLK]
                )
            for kb in range(NKB):
                nc.sync.dma_start(out=vt_tiles[kb], in_=v_r[:, kb * 4:(kb + 1) * 4, :])
            for sp in range(1, NSP):
                nc.sync.dma_start(
                    out=qt_tiles[sp], in_=qT_r[:, :, sp * SPQ:(sp + 1) * SPQ]
                )

            for sp in range(NSP):
                qt = qt_tiles[sp]

                # Phase 1: sT[k-tile, q] for all 32 k-tiles, then exp -> bf16.
                # VectorE keeps a running fp32 partition-wise sum of the exp
                # tiles so the softmax denominator needs only ONE tiny matmul
                # per q-subpanel instead of one per (q-subpanel, k-tile).
                exp_tiles = []
                acc = accp.tile([P, SPQ], f32, tag="acc", name=f"acc{sp}")
                for kt_i in range(NKT):
                    ps = ps_s.tile([P, SPQ], f32, tag="ps")
                    kb, ki = divmod(kt_i, 4)
                    for c in range(DC):
                        nc.tensor.matmul(
                            ps,
                            lhsT=kt_tiles[kb][:, c, ki * P:(ki + 1) * P],
                            rhs=qt[:, c, :],
                            start=(c == 0),
                            stop=(c == DC - 1),
                        )
                    e = expp.tile([P, SPQ], bf16, tag="exp")
                    nc.scalar.activation(
                        e, ps, mybir.ActivationFunctionType.Exp, scale=1.0 / 32.0
                    )
                    if kt_i == 0:
                        nc.vector.tensor_copy(out=acc, in_=e)
                    else:
                        nc.vector.tensor_add(acc, acc, e)
                    exp_tiles.append(e)

                # Phase 2: out[q, d] = exp^T @ V, per 128-query subpanel
                for qs in range(NQS):
                    po0 = ps_o.tile([P, 512], f32, tag="po")
                    po1 = ps_o.tile([P, 512], f32, tag="po")
                    pd = ps_d.tile([P, 1], f32, tag="pd")
                    # den[q] = sum_p acc[p, q] up front - acc is ready since
                    # phase 1, so the reciprocal overlaps the k-loop below
                    # instead of sitting on the tail critical path.
                    nc.tensor.matmul(
                        pd, lhsT=acc[:, qs * P:(qs + 1) * P], rhs=ones32[:, 0:1],
                        start=True, stop=True,
                    )
                    r = small.tile([P, 1], f32, tag="recip")
                    nc.vector.reciprocal(r, pd)
                    for kt_i in range(NKT):
                        w = exp_tiles[kt_i][:, qs * P:(qs + 1) * P]
                        kb, ki = divmod(kt_i, 4)
                        st = kt_i == 0
                        fin = kt_i == NKT - 1
                        nc.tensor.matmul(
                            po0, lhsT=w, rhs=vt_tiles[kb][:, ki, 0:512],
                            start=st, stop=fin,
                        )
                        nc.tensor.matmul(
                            po1, lhsT=w, rhs=vt_tiles[kb][:, ki, 512:1024],
                            start=st, stop=fin,
                        )
                    o = outp.tile([P, D], f32, tag="o")
                    # Wait-absorber: the TensorScalar ISA struct only fits one
                    # sem wait, so take the WAR-vs-previous-store wait on a
                    # cheap DVE memset first (WAW on o then orders via FIFO).
                    nc.vector.memset(o[:, 0:1], 0.0)
                    row = (sp * NQS + qs) * P
                    # Split normalize+store so the first half's SWDGE store
                    # overlaps the second half's TensorScalar on the tail.
                    nc.vector.tensor_scalar_mul(o[:, 0:512], po0, r)
                    nc.gpsimd.dma_start(out=out[row:row + P, 0:512], in_=o[:, 0:512])
                    nc.vector.tensor_scalar_mul(o[:, 512:1024], po1, r)
                    nc.gpsimd.dma_start(
                        out=out[row:row + P, 512:1024], in_=o[:, 512:1024]
                    )

    _split_excess_waits(nc, mybir)
    return nc


def _split_excess_waits(nc, mybir):
    """This walrus build only fits ONE embedded sem wait per engine
    instruction (setupSyncWait: 'Too many sync wait commands'). Hoist all
    but one wait of each engine instruction onto standalone EventSemaphore
    instructions (the same thing wait_ge() emits) right before it in the
    same engine stream - semantically identical, engine streams are FIFO.
    SWDGE (Pool) DMACopy waits are executed by GPSIMD ucode and tolerate
    multiple waits; leave DMAs alone."""
    for fn in nc.m.functions:
        for bb in fn.blocks:
            new_insts = []
            changed = False
            for ins in bb.instructions:
                si = ins.sync_info
                if (
                    si is not None
                    and ins.opcode != "EventSemaphore"
                    and len(si.on_wait) > 1
                ):
                    waits = list(si.on_wait)
                    for j, w in enumerate(waits[:-1]):
                        new_insts.append(
                            mybir.InstEventSemaphore(
                                name=f"{ins.name}-xw{j}",
                                engine=ins.engine,
                                ins=[],
                                outs=[],
                                sync_info=mybir.SyncInfo(on_wait=[w], on_update=[]),
                            )
                        )
                    si.on_wait = [waits[-1]]
                    changed = True
                new_insts.append(ins)
            if changed:
                bb.instructions = new_insts


def get_nc():
    if "nc" not in _BUILT:
        _BUILT["nc"] = _build_nc()
    return _BUILT["nc"]


def kernel(query, key, value):
    from concourse.bass_utils import run_bass_kernel_spmd

    q = np.asarray(query)
    k = np.asarray(key)
    v = np.asarray(value)
    B, S, Dm = q.shape
    assert (B, S, Dm) == (4, 4096, 1024)
    bf = ml_dtypes.bfloat16

    in_maps = []
    for core in range(8):
        b, h = divmod(core, 2)
        qs = q[b, h * SQ:(h + 1) * SQ, :]
        in_maps.append({
            "qT": np.ascontiguousarray(qs.T).astype(bf),
            "kT": np.ascontiguousarray(k[b].T).astype(bf),
            "v": np.ascontiguousarray(v[b]).astype(bf),
        })

    nc = get_nc()
    res = run_bass_kernel_spmd(nc, in_maps, list(range(8)))

    out = np.empty((B, S, Dm), np.float32)
    for core in range(8):
        b, h = divmod(core, 2)
        out[b, h * SQ:(h + 1) * SQ, :] = res.results[core]["out"]
    return out


# revision 26
# speedup vs baseline: 1.0503x; 1.0050x over previous
"""Multi-head attention (single head, d_model=1024) Bass kernel for 8 trn2 cores.

Problem: query/key/value [4, 4096, 1024] fp32 ->
    out = softmax(Q K^T / sqrt(1024)) V   per batch.

Sharding: pure data parallel. 2 cores per batch element; each core handles
2048 query rows against the batch's full K/V (sequence-parallel over query
rows with full K/V replication).

Per-core device kernel (all matmuls bf16 inputs, fp32 PSUM accumulation):
  inputs (host pre-transposed/cast): qT [1024, 2048], kT [1024, 4096],
  v [4096, 1024], all bf16.
  Phase 1 computes scores TRANSPOSED: sT[k, q] = sum_d kT[d,k] * qT[d,q],
  so that softmax weights land in [k, q] layout == exactly the lhsT layout
  the second matmul needs, and the output comes out in natural [q, d] layout.
  No max subtraction: scores ~ N(0,1) (scale 1/32 folded into ACT's
  free affine), exp cannot overflow.
  Denominators via an extra N=1 matmul against a ones column per q-panel:
  den[q] = sum_k exp[k, q]; final normalize on VectorE with per-partition
  reciprocal.
"""

import numpy as np
import ml_dtypes

P = 128
D = 1024
DC = D // P          # 8 contraction chunks
SK = 4096
NKT = SK // P        # 32 k-tiles
SQ = 2048            # query rows per core
SPQ = 512            # superpanel query width (phase-1 matmul free dim)
NSP = SQ // SPQ      # 4 superpanels
NQS = SPQ // P       # 4 q-subpanels per superpanel
KBLK = 512           # kT/v DMA block: 512 k rows (4 k-tiles)
NKB = SK // KBLK     # 8 dma blocks

_BUILT = {}


def _build_nc():
    import concourse.bass as bass
    import concourse.tile as tile
    from concourse import mybir

    bf16 = mybir.dt.bfloat16
    f32 = mybir.dt.float32

    nc = bass.Bass()
    qT = nc.declare_dram_parameter("qT", [D, SQ], bf16, isOutput=False)
    kT = nc.declare_dram_parameter("kT", [D, SK], bf16, isOutput=False)
    v = nc.declare_dram_parameter("v", [SK, D], bf16, isOutput=False)
    out = nc.declare_dram_parameter("out", [SQ, D], f32, isOutput=True)

    # DRAM views with the partition dim innermost-of-128:
    qT_r = qT[:].rearrange("(c p) q -> p c q", p=P)      # [128, 8, 2048]
    kT_r = kT[:].rearrange("(c p) k -> p c k", p=P)      # [128, 8, 4096]
    v_r = v[:].rearrange("(n p) d -> p n d", p=P)        # [128, 32, 1024]

    with tile.TileContext(nc) as tc:
        from contextlib import ExitStack

        with ExitStack() as ctx:
            wpool = ctx.enter_context(tc.tile_pool(name="wpool", bufs=1))
            qpool = ctx.enter_context(tc.tile_pool(name="qpool", bufs=2))
            expp = ctx.enter_context(tc.tile_pool(name="expp", bufs=NKT + 1))
            outp = ctx.enter_context(tc.tile_pool(name="outp", bufs=3))
            small = ctx.enter_context(tc.tile_pool(name="small", bufs=4))
            ps_s = ctx.enter_context(tc.tile_pool(name="ps_s", bufs=3, space="PSUM"))
            ps_o = ctx.enter_context(tc.tile_pool(name="ps_o", bufs=4, space="PSUM"))
            ps_d = ctx.enter_context(tc.tile_pool(name="ps_d", bufs=1, space="PSUM"))

            ones = wpool.tile([P, 1], bf16, tag="ones")
            nc.vector.memset(ones, 1.0)
            ones32 = wpool.tile([P, 1], f32, tag="ones32")
            nc.vector.memset(ones32, 1.0)
            accp = ctx.enter_context(tc.tile_pool(name="accp", bufs=2))

            # HAM warm-up: dummy back-to-back matmuls on a zeroed tile keep
            # the PE activity monitor busy while the first input DMAs land,
            # so the real matmuls start at 2.4 GHz instead of 1.2 GHz.
            warm = wpool.tile([P, 512], bf16, tag="warm")
            nc.vector.memset(warm, 0.0)
            wps = ps_s.tile([P, 512], f32, tag="ps")
            NWARM = 14
            for i in range(NWARM):
                nc.tensor.matmul(
                    wps, lhsT=warm[:, 0:P], rhs=warm,
                    start=(i == 0), stop=(i == NWARM - 1),
                )

            # First K^T block + first superpanel's Q first so PE can start
            # ~11us in, then the rest (phase-1 feed order), V trailing
            # (phase 2 only). qt0 split in half so the first d-chunk matmuls
            # start after 0.5 MB instead of 1 MB.
            qt_tiles = [
                qpool.tile([P, DC, SPQ], bf16, tag="qt", name=f"qt{i}")
                for i in range(NSP)
            ]
            kt_tiles = []
            vt_tiles = []
            for kb in range(NKB):
                kt_tiles.append(wpool.tile([P, DC, KBLK], bf16, tag=f"kT{kb}",
                                           name=f"kTs{kb}"))
                vt_tiles.append(wpool.tile([P, 4, D], bf16, tag=f"v{kb}",
                                           name=f"vs{kb}"))
            nc.sync.dma_start(out=kt_tiles[0], in_=kT_r[:, :, 0:KBLK])
            nc.sync.dma_start(out=qt_tiles[0][:, 0:4, :], in_=qT_r[:, 0:4, 0:SPQ])
            nc.sync.dma_start(out=qt_tiles[0][:, 4:8, :], in_=qT_r[:, 4:8, 0:SPQ])
            for kb in range(1, NKB):
                nc.sync.dma_start(
                    out=kt_tiles[kb], in_=kT_r[:, :, kb * KBLK:(kb + 1) * KBLK]
                )
            for kb in range(NKB):
                nc.sync.dma_start(out=vt_tiles[kb], in_=v_r[:, kb * 4:(kb + 1) * 4, :])
            for sp in range(1, NSP):
                nc.sync.dma_start(
                    out=qt_tiles[sp], in_=qT_r[:, :, sp * SPQ:(sp + 1) * SPQ]
                )

            for sp in range(NSP):
                qt = qt_tiles[sp]

                # Phase 1: sT[k-tile, q] for all 32 k-tiles, then exp -> bf16.
                # VectorE keeps a running fp32 partition-wise sum of the exp
                # tiles so the softmax denominator needs only ONE tiny matmul
                # per q-subpanel instead of one per (q-subpanel, k-tile).
                exp_tiles = []
                acc = accp.tile([P, SPQ], f32, tag="acc", name=f"acc{sp}")
                for kt_i in range(NKT):
                    ps = ps_s.tile([P, SPQ], f32, tag="ps")
                    kb, ki = divmod(kt_i, 4)
                    for c in range(DC):
                        nc.tensor.matmul(
                            ps,
                            lhsT=kt_tiles[kb][:, c, ki * P:(ki + 1) * P],
                            rhs=qt[:, c, :],
                            start=(c == 0),
                            stop=(c == DC - 1),
                        )
                    e = expp.tile([P, SPQ], bf16, tag="exp")
                    nc.scalar.activation(
                        e, ps, mybir.ActivationFunctionType.Exp, scale=1.0 / 32.0
                    )
                    if kt_i == 0:
                        nc.vector.tensor_copy(out=acc, in_=e)
                    else:
                        nc.vector.tensor_add(acc, acc, e)
                    exp_tiles.append(e)

                # Phase 2: out[q, d] = exp^T @ V, per 128-query subpanel
                for qs in range(NQS):
                    po0 = ps_o.tile([P, 512], f32, tag="po")
                    po1 = ps_o.tile([P, 512], f32, tag="po")
                    pd = ps_d.tile([P, 1], f32, tag="pd")
                    for kt_i in range(NKT):
                        w = exp_tiles[kt_i][:, qs * P:(qs + 1) * P]
                        kb, ki = divmod(kt_i, 4)
                        st = kt_i == 0
                        fin = kt_i == NKT - 1
                        nc.tensor.matmul(
                            po0, lhsT=w, rhs=vt_tiles[kb][:, ki, 0:512],
                            start=st, stop=fin,
                        )
                        nc.tensor.matmul(
                            po1, lhsT=w, rhs=vt_tiles[kb][:, ki, 512:1024],
                            start=st, stop=fin,
                        )
                    # den[q] = sum_p acc[p, q] in one fp32 N=1 matmul
                    nc.tensor.matmul(
                        pd, lhsT=acc[:, qs * P:(qs + 1) * P], rhs=ones32[:, 0:1],
                        start=True, stop=True,
                    )
                    o = outp.tile([P, D], f32, tag="o")
                    # Wait-absorber: the TensorScalar ISA struct only fits one
                    # sem wait, so take the WAR-vs-previous-store wait on a
                    # cheap DVE memset first (WAW on o then orders via FIFO).
                    nc.vector.memset(o[:, 0:1], 0.0)
                    r = small.tile([P, 1], f32, tag="recip")
                    nc.vector.reciprocal(r, pd)
                    nc.vector.tensor_scalar_mul(o[:, 0:512], po0, r)
                    nc.vector.tensor_scalar_mul(o[:, 512:1024], po1, r)
                    row = (sp * NQS + qs) * P
                    # SWDGE: the HWDGE DIRECT2D pseudo only fits 2 sem waits
                    # and these stores need RAW(VE) + queue-order waits.
                    nc.gpsimd.dma_start(out=out[row:row + P, :], in_=o)

    _split_excess_waits(nc, mybir)
    return nc


def _split_excess_waits(nc, mybir):
    """This walrus build only fits ONE embedded sem wait per engine
    instruction (setupSyncWait: 'Too many sync wait commands'). Hoist all
    but one wait of each engine instruction onto standalone EventSemaphore
    instructions (the same thing wait_ge() emits) right before it in the
    same engine stream - semantically identical, engine streams are FIFO.
    SWDGE (Pool) DMACopy waits are executed by GPSIMD ucode and tolerate
    multiple waits; leave DMAs alone."""
    for fn in nc.m.functions:
        for bb in fn.blocks:
            new_insts = []
            changed = False
            for ins in bb.instructions:
                si = ins.sync_info
                if (
                    si is not None
                    and ins.opcode != "EventSemaphore"
                    and len(si.on_wait) > 1
                ):
                    waits = list(si.on_wait)
                    for j, w in enumerate(waits[:-1]):
                        new_insts.append(
                            mybir.InstEventSemaphore(
                                name=f"{ins.name}-xw{j}",
                                engine=ins.engine,
                                ins=[],
                                outs=[],
                                sync_info=mybir.SyncInfo(on_wait=[w], on_update=[]),
                            )
                        )
                    si.on_wait = [waits[-1]]
                    changed = True
                new_insts.append(ins)
            if changed:
                bb.instructions = new_insts


def get_nc():
    if "nc" not in _BUILT:
        _BUILT["nc"] = _build_nc()
    return _BUILT["nc"]


def kernel(query, key, value):
    from concourse.bass_utils import run_bass_kernel_spmd

    q = np.asarray(query)
    k = np.asarray(key)
    v = np.asarray(value)
    B, S, Dm = q.shape
    assert (B, S, Dm) == (4, 4096, 1024)
    bf = ml_dtypes.bfloat16

    in_maps = []
    for core in range(8):
        b, h = divmod(core, 2)
        qs = q[b, h * SQ:(h + 1) * SQ, :]
        in_maps.append({
            "qT": np.ascontiguousarray(qs.T).astype(bf),
            "kT": np.ascontiguousarray(k[b].T).astype(bf),
            "v": np.ascontiguousarray(v[b]).astype(bf),
        })

    nc = get_nc()
    res = run_bass_kernel_spmd(nc, in_maps, list(range(8)))

    out = np.empty((B, S, Dm), np.float32)
    for core in range(8):
        b, h = divmod(core, 2)
        out[b, h * SQ:(h + 1) * SQ, :] = res.results[core]["out"]
    return out
